# revision 25
# baseline (speedup 1.0000x reference)
"""KNN (k=16, 10 classes) on 8 Trainium2 NeuronCores via Bass/Tile.

Distributed ANN: shard X_train across 8 cores; each core scores its shard
against all 4096 queries and returns per-class-window top-8 candidates;
host merges to global top-16 and votes.

Scores v[q, j] = 2<t_q, x_j> - ||x_j||^2 are computed in 3 fp16 matmul
passes per 512-col PSUM bank (fp16 streams at 1 cycle/row vs 4 for fp32,
and 2-byte weights pipeline their LDWEIGHTS):
  pass1: q_h . x_h          (all 128 dims; fp16 products are exact in
  pass2: q_l . x_h [0:127]   fp32 PSUM accumulation)
         + row127 borrowed to add -||x||^2 hi (fp16)
  pass3: q_h . x_l [0:127]
         + row127 borrowed to add -||x||^2 lo (fp16)
where q = 2*X_test = q_h + q_l and x = x_h + x_l are fp16 hi/lo splits.
Dropped terms (q_l.x_l, dim-127 residual cross terms) give ~6e-4 rms
score error vs fp64 -- ~30x better than 1-pass fp32r, 0-1 vote flips.

Train layout per core: for each class c exactly s_c rows (identical s_c
on all cores; shortfall padded with dummy rows scoring -6e4) so the SPMD
program's class-window scan offsets are the same on every core, and a
candidate's position identifies its label.  DVE max8 scans each class
window (split only at PSUM ring wrap points) -> top-8 values per
(query, window); candidates accumulate in SBUF, one bulk DMA out, host
merges 8 cores x ~13 windows x 8 values per query -> top-16 -> majority
vote (argmax ties -> smallest label, matching the reference).
"""

import numpy as np

NCORES = 8
D = 128
QTILE = 128
NUM_CLASSES = 10
BANK = 512
RING_BANKS = 8
RING = BANK * RING_BANKS  # 4096
PAD_XXH = np.float16(-60000.0)

_compiled_cache = {}


def _class_sizes(y):
    """Per-core per-class allocation s_c (identical across cores)."""
    n = np.bincount(y, minlength=NUM_CLASSES)
    s = [(int(nc) + NCORES - 1) // NCORES for nc in n]
    # adjust so every window boundary b has b%512 in {0} U [8, 504]
    # (guarantees every ring-split scan piece is >= 8 wide for max8)
    off = 0
    out = []
    for c in range(NUM_CLASSES):
        sc = max(s[c], 8)
        while True:
            r = (off + sc) % BANK
            if r == 0 or 8 <= r <= 504:
                break
            sc += 1
        out.append(sc)
        off += sc
    return out


def _layout(y):
    s = _class_sizes(y)
    tot = sum(s)
    Lp = ((tot + BANK - 1) // BANK) * BANK
    offs = np.concatenate([[0], np.cumsum(s)])  # class windows [offs[c], offs[c+1])
    return s, offs, Lp


def _pieces_for_qtile(qt, offs, Lp, nbanks):
    """Scan pieces for one query tile: class windows split at ring wraps.

    Tail padding past offs[-1] is never scanned (its scores are garbage-
    free anyway, but skipping it saves DVE cycles).

    Returns list of (col_off, length, class)."""
    x0 = (-nbanks * qt * BANK) % RING
    splits = set(range(x0, Lp + 1, RING))
    pieces = []
    for c in range(NUM_CLASSES):
        o, e = int(offs[c]), int(offs[c + 1])
        cuts = sorted([o, e] + [x for x in splits if o < x < e])
        for a, b in zip(cuts[:-1], cuts[1:]):
            pieces.append((a, b - a, c))
    return pieces


def _build_program(NQ, Lp, offs):
    import concourse.bacc as bacc
    import concourse.tile as tile
    import concourse.mybir as mybir

    nbanks = Lp // BANK
    ctot = int(offs[-1])  # real content columns; the rest is tail padding
    nqt = NQ // QTILE

    all_pieces = [_pieces_for_qtile(qt, offs, Lp, nbanks) for qt in range(nqt)]
    slot_base = np.cumsum([0] + [len(p) for p in all_pieces])
    STOT = int(slot_base[-1])

    f16 = mybir.dt.float16
    nc = bacc.Bacc("TRN2", target_bir_lowering=False, debug=False)
    qh = nc.dram_tensor("qh", [D, NQ], f16, kind="ExternalInput")
    s2 = nc.dram_tensor("s2", [D, NQ], f16, kind="ExternalInput")
    s3 = nc.dram_tensor("s3", [D, NQ], f16, kind="ExternalInput")
    m1 = nc.dram_tensor("m1", [D, Lp], f16, kind="ExternalInput")
    m3 = nc.dram_tensor("m3", [D, Lp], f16, kind="ExternalInput")
    xxr = nc.dram_tensor("xxr", [2, Lp], f16, kind="ExternalInput")
    out_cand = nc.dram_tensor("out_cand", [QTILE, STOT * 8], mybir.dt.float32,
                              kind="ExternalOutput")

    GROUP = 4  # banks per stationary-operand group (2*GROUP <= RING_BANKS)

    with tile.TileContext(nc) as tc:
        with tc.tile_pool(name="res", bufs=1) as res, \
             tc.tile_pool(name="ring", bufs=1, space="PSUM") as ringp:
            qh_t = res.tile([D, NQ], f16)
            s2_t = res.tile([D, NQ], f16)
            s3_t = res.tile([D, NQ], f16)
            m1_t = res.tile([D, Lp], f16)
            m2_t = res.tile([D, Lp], f16)
            m3_t = res.tile([D, Lp], f16)
            cand_t = res.tile([QTILE, STOT * 8], mybir.dt.float32)
            # load order matters: the first matmul group needs qtile 0's
            # stationary slices and the first bank chunk of m1/m2/m3, so
            # issue those first, then stream the rest.
            QSPLIT = 8
            qs = NQ // QSPLIT
            NSPLIT = 8
            cs = (Lp // NSPLIT + BANK - 1) // BANK * BANK

            def qsl_i(i):
                return slice(i * qs, (i + 1) * qs)

            def msl_i(i):
                return slice(i * cs, min((i + 1) * cs, Lp))

            for t, d in ((qh_t, qh), (s2_t, s2), (s3_t, s3)):
                nc.sync.dma_start(out=t[:, qsl_i(0)], in_=d[:, qsl_i(0)])
            # m2 rows 0..126 are identical to m1's: don't ship them.  DMA
            # m1/m3 (rows 0..126 of m3; row 127 comes from the xx rows) and
            # replicate m1 -> m2 on the otherwise-idle DVE.
            nc.sync.dma_start(out=m2_t[D - 1:D, :], in_=xxr[0:1, :])
            nc.sync.dma_start(out=m3_t[D - 1:D, :], in_=xxr[1:2, :])
            for i in range(NSPLIT):
                sl = msl_i(i)
                if sl.start < Lp:
                    nc.sync.dma_start(out=m1_t[:, sl], in_=m1[:, sl])
                    nc.sync.dma_start(out=m3_t[0:D - 1, sl], in_=m3[0:D - 1, sl])
                    nc.vector.tensor_copy(m2_t[0:D - 1, sl], m1_t[0:D - 1, sl])
            for i in range(1, QSPLIT):
                for t, d in ((qh_t, qh), (s2_t, s2), (s3_t, s3)):
                    nc.sync.dma_start(out=t[:, qsl_i(i)], in_=d[:, qsl_i(i)])

            ring = ringp.tile([QTILE, RING], mybir.dt.float32)

            # warm the PE's HAM clock gate while input DMAs are in flight:
            # junk matmuls on a never-written SBUF tile have no data deps,
            # so they run immediately and keep the PE at full clock by the
            # time real work starts (values are overwritten by start=True).
            junk = res.tile([D, BANK], f16)
            nc.vector.memset(junk[:], 0.0)
            for w in range(24):
                nc.tensor.matmul(ring[:, (w % RING_BANKS) * BANK:
                                       (w % RING_BANKS) * BANK + BANK],
                                 lhsT=junk[:, 0:QTILE], rhs=junk[:],
                                 start=True, stop=True)

            for qt in range(nqt):
                qsl = slice(qt * QTILE, (qt + 1) * QTILE)
                pieces = all_pieces[qt]
                base = int(slot_base[qt])
                emitted = 0

                def emit_scans(done_cols, emitted):
                    while emitted < len(pieces):
                        o, ln, _c = pieces[emitted]
                        if o + ln > done_cols:
                            break
                        ro = (qt * nbanks * BANK + o) % RING
                        slot = (base + emitted) * 8
                        nc.vector.max(out=cand_t[:, slot:slot + 8],
                                      in_=ring[:, ro:ro + ln])
                        emitted += 1
                    return emitted

                for g0 in range(0, nbanks, GROUP):
                    banks = range(g0, min(g0 + GROUP, nbanks))

                    def rsl(b):
                        s = ((qt * nbanks + b) % RING_BANKS) * BANK
                        return slice(s, s + BANK)

                    def csl(b):
                        return slice(b * BANK, (b + 1) * BANK)

                    for b in banks:
                        nc.tensor.matmul(ring[:, rsl(b)], lhsT=qh_t[:, qsl],
                                         rhs=m1_t[:, csl(b)],
                                         start=True, stop=False)
                    for b in banks:
                        nc.tensor.matmul(ring[:, rsl(b)], lhsT=s2_t[:, qsl],
                                         rhs=m2_t[:, csl(b)],
                                         start=False, stop=False)
                    for b in banks:
                        nc.tensor.matmul(ring[:, rsl(b)], lhsT=s3_t[:, qsl],
                                         rhs=m3_t[:, csl(b)],
                                         start=False, stop=True)
                    emitted = emit_scans(min(g0 + GROUP, nbanks) * BANK, emitted)

                # stream candidates out in tapered batches: big batches
                # early, single qtiles at the end so the final flush (and
                # its descriptor-generation latency) is tiny
                flush = {7: 0, 15: 8, 23: 16, 27: 24, 29: 28, 30: 30, 31: 31}
                if qt in flush:
                    lo8 = int(slot_base[flush[qt]]) * 8
                    hi8 = int(slot_base[qt + 1]) * 8
                    nc.sync.dma_start(out=out_cand[:, lo8:hi8],
                                      in_=cand_t[:, lo8:hi8])

    nc.compile()
    return nc, all_pieces, slot_base


def _get_program(NQ, Lp, offs):
    key = (NQ, Lp, tuple(int(o) for o in offs))
    if key not in _compiled_cache:
        _compiled_cache[key] = _build_program(NQ, Lp, offs)
    return _compiled_cache[key]


def prepare(X_train, y_train, X_test):
    X_train = np.ascontiguousarray(np.asarray(X_train, dtype=np.float32))
    X_test = np.ascontiguousarray(np.asarray(X_test, dtype=np.float32))
    y = np.asarray(y_train).astype(np.int64)
    N, Dd = X_train.shape
    NQ = X_test.shape[0]
    assert Dd == D

    s, offs, Lp = _layout(y)

    order = np.argsort(y, kind="stable")
    n = np.bincount(y, minlength=NUM_CLASSES)
    cstart = np.concatenate([[0], np.cumsum(n)])

    xx = np.einsum("ij,ij->i", X_train.astype(np.float64),
                   X_train.astype(np.float64)).astype(np.float32)

    # per-core fp16 hi/lo train splits, class-contiguous layout
    xh = np.zeros((NCORES, D, Lp), np.float16)
    xl = np.zeros((NCORES, D, Lp), np.float16)
    xxh = np.full((NCORES, Lp), PAD_XXH, np.float16)
    xxl = np.zeros((NCORES, Lp), np.float16)
    for c in range(NUM_CLASSES):
        rows = order[cstart[c]:cstart[c + 1]]
        sc = s[c]
        o = int(offs[c])
        for k in range(NCORES):
            sub = rows[k * sc:(k + 1) * sc]
            m = len(sub)
            if m:
                xt32 = X_train[sub].T  # [D, m] fp32
                h = xt32.astype(np.float16)
                xh[k, :, o:o + m] = h
                xl[k, :, o:o + m] = (xt32 - h.astype(np.float32)).astype(np.float16)
                nx = -xx[sub]
                nh = nx.astype(np.float16)
                xxh[k, o:o + m] = nh
                xxl[k, o:o + m] = (nx - nh.astype(np.float32)).astype(np.float16)

    # moving tensors: m1 = xh; m2 is replicated from m1 on device with
    # row 127 := xxh (shipped via xxr); m3 = xl with row 127 := xxl
    xxr = np.stack([xxh, xxl], axis=1)  # [NCORES, 2, Lp]

    # stationary tensors: qh = fp16(2*X_test)^T; s2 = [q_l rows; ones];
    # s3 = [q_h rows; ones]
    q32 = (2.0 * X_test).T.astype(np.float32)          # [D, NQ]
    qh = q32.astype(np.float16)
    ql = (q32 - qh.astype(np.float32)).astype(np.float16)
    s2m = ql.copy()
    s2m[D - 1, :] = np.float16(1.0)
    s3m = qh.copy()
    s3m[D - 1, :] = np.float16(1.0)

    nc, all_pieces, slot_base = _get_program(NQ, Lp, offs)

    in_maps = [{"qh": qh, "s2": s2m, "s3": s3m,
                "m1": np.ascontiguousarray(xh[k]),
                "m3": np.ascontiguousarray(xl[k]),
                "xxr": np.ascontiguousarray(xxr[k])}
               for k in range(NCORES)]

    slot_labels = np.concatenate(
        [np.repeat([c for (_o, _l, c) in pieces], 8) for pieces in all_pieces])
    meta = (all_pieces, slot_base, slot_labels, NQ)
    return nc, in_maps, meta


def merge(results, meta):
    all_pieces, slot_base, slot_labels, NQ = meta
    nqt = NQ // QTILE
    K = 16
    vals = np.stack([results[k]["out_cand"] for k in range(NCORES)], axis=0)
    preds = np.empty(NQ, np.int64)
    for qt in range(nqt):
        lo = int(slot_base[qt]) * 8
        hi = int(slot_base[qt + 1]) * 8
        v = vals[:, :, lo:hi]                        # [NCORES, QTILE, W]
        lab = slot_labels[lo:hi]
        v = np.moveaxis(v, 1, 0).reshape(QTILE, -1)  # [QTILE, NCORES*W]
        labs = np.tile(lab, NCORES)
        sel = np.argpartition(-v, K - 1, axis=1)[:, :K]
        top_lab = labs[sel]
        counts = np.zeros((QTILE, NUM_CLASSES), np.int64)
        for c in range(NUM_CLASSES):
            counts[:, c] = (top_lab == c).sum(1)
        preds[qt * QTILE:(qt + 1) * QTILE] = counts.argmax(1)
    return preds.astype(np.int64)


def kernel(X_train, y_train, X_test):
    from concourse.bass_utils import run_bass_kernel_spmd
    nc, in_maps, meta = prepare(X_train, y_train, X_test)
    res = run_bass_kernel_spmd(nc, in_maps, core_ids=list(range(NCORES)))
    return merge(res.results, meta)


# revision 26
# speedup vs baseline: 1.1914x; 1.1914x over previous
"""KNN (k=16, 10 classes) on 8 Trainium2 NeuronCores via Bass/Tile.

Distributed ANN: shard X_train across 8 cores; each core scores its shard
against all 4096 queries and returns per-class-window top-8 candidates;
host merges to global top-16 and votes.

Scores v[q, j] = 2<t_q, x_j> - ||x_j||^2 are computed in 3 fp16 matmul
passes per 512-col PSUM bank (fp16 streams at 1 cycle/row vs 4 for fp32,
and 2-byte weights pipeline their LDWEIGHTS):
  pass1: q_h . x_h          (all 128 dims; fp16 products are exact in
  pass2: q_l . x_h [0:127]   fp32 PSUM accumulation)
         + row127 borrowed to add -||x||^2 hi (fp16)
  pass3: q_h . x_l [0:127]
         + row127 borrowed to add -||x||^2 lo (fp16)
where q = 2*X_test = q_h + q_l and x = x_h + x_l are fp16 hi/lo splits.
Dropped terms (q_l.x_l, dim-127 residual cross terms) give ~6e-4 rms
score error vs fp64 -- ~30x better than 1-pass fp32r, 0-1 vote flips.

Train layout per core: for each class c exactly s_c rows (identical s_c
on all cores; shortfall padded with dummy rows scoring -6e4) so the SPMD
program's class-window scan offsets are the same on every core, and a
candidate's position identifies its label.  DVE max8 scans each class
window (split only at PSUM ring wrap points) -> top-8 values per
(query, window); candidates accumulate in SBUF, one bulk DMA out, host
merges 8 cores x ~13 windows x 8 values per query -> top-16 -> majority
vote (argmax ties -> smallest label, matching the reference).
"""

import numpy as np

NCORES = 8
D = 128
QTILE = 128
NUM_CLASSES = 10
BANK = 512
RING_BANKS = 8
RING = BANK * RING_BANKS  # 4096
PAD_XXH = np.float16(-60000.0)

_compiled_cache = {}


def _class_sizes(y):
    """Per-core per-class allocation s_c (identical across cores)."""
    n = np.bincount(y, minlength=NUM_CLASSES)
    s = [(int(nc) + NCORES - 1) // NCORES for nc in n]
    # adjust so every window boundary b has b%512 in {0} U [8, 504]
    # (guarantees every ring-split scan piece is >= 8 wide for max8)
    off = 0
    out = []
    for c in range(NUM_CLASSES):
        sc = max(s[c], 8)
        while True:
            r = (off + sc) % BANK
            if r == 0 or 8 <= r <= 504:
                break
            sc += 1
        out.append(sc)
        off += sc
    return out


def _layout(y):
    s = _class_sizes(y)
    tot = sum(s)
    Lp = ((tot + BANK - 1) // BANK) * BANK
    offs = np.concatenate([[0], np.cumsum(s)])  # class windows [offs[c], offs[c+1])
    return s, offs, Lp


def _pieces_for_qtile(qt, offs, Lp, nbanks):
    """Scan pieces for one query tile: class windows split at ring wraps.

    Tail padding past offs[-1] is never scanned (its scores are garbage-
    free anyway, but skipping it saves DVE cycles).

    Returns list of (col_off, length, class)."""
    x0 = (-nbanks * qt * BANK) % RING
    splits = set(range(x0, Lp + 1, RING))
    pieces = []
    for c in range(NUM_CLASSES):
        o, e = int(offs[c]), int(offs[c + 1])
        cuts = sorted([o, e] + [x for x in splits if o < x < e])
        for a, b in zip(cuts[:-1], cuts[1:]):
            pieces.append((a, b - a, c))
    return pieces


def _build_program(NQ, Lp, offs):
    import concourse.bacc as bacc
    import concourse.tile as tile
    import concourse.mybir as mybir

    nbanks = Lp // BANK
    ctot = int(offs[-1])  # real content columns; the rest is tail padding
    nqt = NQ // QTILE

    all_pieces = [_pieces_for_qtile(qt, offs, Lp, nbanks) for qt in range(nqt)]
    slot_base = np.cumsum([0] + [len(p) for p in all_pieces])
    STOT = int(slot_base[-1])

    f16 = mybir.dt.float16
    nc = bacc.Bacc("TRN2", target_bir_lowering=False, debug=False)
    qh = nc.dram_tensor("qh", [D, NQ], f16, kind="ExternalInput")
    s2 = nc.dram_tensor("s2", [D, NQ], f16, kind="ExternalInput")
    s3 = nc.dram_tensor("s3", [D, NQ], f16, kind="ExternalInput")
    m1 = nc.dram_tensor("m1", [D, Lp], f16, kind="ExternalInput")
    m2 = nc.dram_tensor("m2", [D, Lp], f16, kind="ExternalInput")
    m3 = nc.dram_tensor("m3", [D, Lp], f16, kind="ExternalInput")
    out_cand = nc.dram_tensor("out_cand", [QTILE, STOT * 8], mybir.dt.float32,
                              kind="ExternalOutput")

    GROUP = 4  # banks per stationary-operand group (2*GROUP <= RING_BANKS)

    with tile.TileContext(nc) as tc:
        with tc.tile_pool(name="res", bufs=1) as res, \
             tc.tile_pool(name="ring", bufs=1, space="PSUM") as ringp:
            qh_t = res.tile([D, NQ], f16)
            s2_t = res.tile([D, NQ], f16)
            s3_t = res.tile([D, NQ], f16)
            m1_t = res.tile([D, Lp], f16)
            m2_t = res.tile([D, Lp], f16)
            m3_t = res.tile([D, Lp], f16)
            cand_t = res.tile([QTILE, STOT * 8], mybir.dt.float32)
            # load order matters: the first matmul group needs qtile 0's
            # stationary slices and the first bank chunk of m1/m2/m3, so
            # issue those first, then stream the rest.
            QSPLIT = 8
            qs = NQ // QSPLIT
            NSPLIT = 8
            cs = (Lp // NSPLIT + BANK - 1) // BANK * BANK

            def qsl_i(i):
                return slice(i * qs, (i + 1) * qs)

            def msl_i(i):
                return slice(i * cs, min((i + 1) * cs, Lp))

            for t, d in ((qh_t, qh), (s2_t, s2), (s3_t, s3)):
                nc.sync.dma_start(out=t[:, qsl_i(0)], in_=d[:, qsl_i(0)])
            for t, d in ((m1_t, m1), (m2_t, m2), (m3_t, m3)):
                nc.sync.dma_start(out=t[:, msl_i(0)], in_=d[:, msl_i(0)])
            for i in range(1, NSPLIT):
                sl = msl_i(i)
                if sl.start < Lp:
                    for t, d in ((m1_t, m1), (m2_t, m2), (m3_t, m3)):
                        nc.sync.dma_start(out=t[:, sl], in_=d[:, sl])
            for i in range(1, QSPLIT):
                for t, d in ((qh_t, qh), (s2_t, s2), (s3_t, s3)):
                    nc.sync.dma_start(out=t[:, qsl_i(i)], in_=d[:, qsl_i(i)])

            ring = ringp.tile([QTILE, RING], mybir.dt.float32)

            # warm the PE's HAM clock gate while input DMAs are in flight:
            # junk matmuls on a never-written SBUF tile have no data deps,
            # so they run immediately and keep the PE at full clock by the
            # time real work starts (values are overwritten by start=True).
            junk = res.tile([D, BANK], f16)
            nc.vector.memset(junk[:], 0.0)
            for w in range(24):
                nc.tensor.matmul(ring[:, (w % RING_BANKS) * BANK:
                                       (w % RING_BANKS) * BANK + BANK],
                                 lhsT=junk[:, 0:QTILE], rhs=junk[:],
                                 start=True, stop=True)

            for qt in range(nqt):
                qsl = slice(qt * QTILE, (qt + 1) * QTILE)
                pieces = all_pieces[qt]
                base = int(slot_base[qt])
                emitted = 0

                def emit_scans(done_cols, emitted):
                    while emitted < len(pieces):
                        o, ln, _c = pieces[emitted]
                        if o + ln > done_cols:
                            break
                        ro = (qt * nbanks * BANK + o) % RING
                        slot = (base + emitted) * 8
                        nc.vector.max(out=cand_t[:, slot:slot + 8],
                                      in_=ring[:, ro:ro + ln])
                        emitted += 1
                    return emitted

                for g0 in range(0, nbanks, GROUP):
                    banks = range(g0, min(g0 + GROUP, nbanks))

                    def rsl(b):
                        s = ((qt * nbanks + b) % RING_BANKS) * BANK
                        return slice(s, s + BANK)

                    def csl(b):
                        return slice(b * BANK, (b + 1) * BANK)

                    for b in banks:
                        nc.tensor.matmul(ring[:, rsl(b)], lhsT=qh_t[:, qsl],
                                         rhs=m1_t[:, csl(b)],
                                         start=True, stop=False)
                    for b in banks:
                        nc.tensor.matmul(ring[:, rsl(b)], lhsT=s2_t[:, qsl],
                                         rhs=m2_t[:, csl(b)],
                                         start=False, stop=False)
                    for b in banks:
                        nc.tensor.matmul(ring[:, rsl(b)], lhsT=s3_t[:, qsl],
                                         rhs=m3_t[:, csl(b)],
                                         start=False, stop=True)
                    emitted = emit_scans(min(g0 + GROUP, nbanks) * BANK, emitted)

                # stream candidates out in tapered batches: big batches
                # early, single qtiles at the end so the final flush (and
                # its descriptor-generation latency) is tiny
                flush = {7: 0, 15: 8, 23: 16, 27: 24, 29: 28, 30: 30, 31: 31}
                if qt in flush:
                    lo8 = int(slot_base[flush[qt]]) * 8
                    hi8 = int(slot_base[qt + 1]) * 8
                    nc.sync.dma_start(out=out_cand[:, lo8:hi8],
                                      in_=cand_t[:, lo8:hi8])

    nc.compile()
    return nc, all_pieces, slot_base


def _get_program(NQ, Lp, offs):
    key = (NQ, Lp, tuple(int(o) for o in offs))
    if key not in _compiled_cache:
        _compiled_cache[key] = _build_program(NQ, Lp, offs)
    return _compiled_cache[key]


def prepare(X_train, y_train, X_test):
    X_train = np.ascontiguousarray(np.asarray(X_train, dtype=np.float32))
    X_test = np.ascontiguousarray(np.asarray(X_test, dtype=np.float32))
    y = np.asarray(y_train).astype(np.int64)
    N, Dd = X_train.shape
    NQ = X_test.shape[0]
    assert Dd == D

    s, offs, Lp = _layout(y)

    order = np.argsort(y, kind="stable")
    n = np.bincount(y, minlength=NUM_CLASSES)
    cstart = np.concatenate([[0], np.cumsum(n)])

    xx = np.einsum("ij,ij->i", X_train.astype(np.float64),
                   X_train.astype(np.float64)).astype(np.float32)

    # per-core fp16 hi/lo train splits, class-contiguous layout
    xh = np.zeros((NCORES, D, Lp), np.float16)
    xl = np.zeros((NCORES, D, Lp), np.float16)
    xxh = np.full((NCORES, Lp), PAD_XXH, np.float16)
    xxl = np.zeros((NCORES, Lp), np.float16)
    for c in range(NUM_CLASSES):
        rows = order[cstart[c]:cstart[c + 1]]
        sc = s[c]
        o = int(offs[c])
        for k in range(NCORES):
            sub = rows[k * sc:(k + 1) * sc]
            m = len(sub)
            if m:
                xt32 = X_train[sub].T  # [D, m] fp32
                h = xt32.astype(np.float16)
                xh[k, :, o:o + m] = h
                xl[k, :, o:o + m] = (xt32 - h.astype(np.float32)).astype(np.float16)
                nx = -xx[sub]
                nh = nx.astype(np.float16)
                xxh[k, o:o + m] = nh
                xxl[k, o:o + m] = (nx - nh.astype(np.float32)).astype(np.float16)

    # moving tensors: m1 = xh; m2 = xh with row 127 := xxh; m3 = xl w/ xxl
    m2 = xh.copy()
    m2[:, D - 1, :] = xxh
    m3 = xl.copy()
    m3[:, D - 1, :] = xxl

    # stationary tensors: qh = fp16(2*X_test)^T; s2 = [q_l rows; ones];
    # s3 = [q_h rows; ones]
    q32 = (2.0 * X_test).T.astype(np.float32)          # [D, NQ]
    qh = q32.astype(np.float16)
    ql = (q32 - qh.astype(np.float32)).astype(np.float16)
    s2m = ql.copy()
    s2m[D - 1, :] = np.float16(1.0)
    s3m = qh.copy()
    s3m[D - 1, :] = np.float16(1.0)

    nc, all_pieces, slot_base = _get_program(NQ, Lp, offs)

    in_maps = [{"qh": qh, "s2": s2m, "s3": s3m,
                "m1": np.ascontiguousarray(xh[k]),
                "m2": np.ascontiguousarray(m2[k]),
                "m3": np.ascontiguousarray(m3[k])}
               for k in range(NCORES)]

    slot_labels = np.concatenate(
        [np.repeat([c for (_o, _l, c) in pieces], 8) for pieces in all_pieces])
    meta = (all_pieces, slot_base, slot_labels, NQ)
    return nc, in_maps, meta


def merge(results, meta):
    all_pieces, slot_base, slot_labels, NQ = meta
    nqt = NQ // QTILE
    K = 16
    vals = np.stack([results[k]["out_cand"] for k in range(NCORES)], axis=0)
    preds = np.empty(NQ, np.int64)
    for qt in range(nqt):
        lo = int(slot_base[qt]) * 8
        hi = int(slot_base[qt + 1]) * 8
        v = vals[:, :, lo:hi]                        # [NCORES, QTILE, W]
        lab = slot_labels[lo:hi]
        v = np.moveaxis(v, 1, 0).reshape(QTILE, -1)  # [QTILE, NCORES*W]
        labs = np.tile(lab, NCORES)
        sel = np.argpartition(-v, K - 1, axis=1)[:, :K]
        top_lab = labs[sel]
        counts = np.zeros((QTILE, NUM_CLASSES), np.int64)
        for c in range(NUM_CLASSES):
            counts[:, c] = (top_lab == c).sum(1)
        preds[qt * QTILE:(qt + 1) * QTILE] = counts.argmax(1)
    return preds.astype(np.int64)


def kernel(X_train, y_train, X_test):
    from concourse.bass_utils import run_bass_kernel_spmd
    nc, in_maps, meta = prepare(X_train, y_train, X_test)
    res = run_bass_kernel_spmd(nc, in_maps, core_ids=list(range(NCORES)))
    return merge(res.results, meta)
